# revision 48
# baseline (speedup 1.0000x reference)
"""MultiHeadAttention (faithful raw-reshape variant) on 8 trn2 NeuronCores.

Math (per batch b):
  Y  = Xq @ Wq.T            [S, D]
  Z  = Xk @ Wk.T            [S, D]
  V  = Xv @ Wv.T            [S, D]
  reshape (B,S,D)->(B,H,S,dk) is a *raw view*: head h <- rows [128h, 128h+128)
  of Y/Z/V; within the block, q = 16t + j maps to (row t, features 64j..64j+64).
  A  = softmax(Qh @ Kh.T / 8), O = A @ Vh, placed back into the same raw view,
  out = Hcat @ Wo.T + b_o.

Heads partition the *rows* of Y/Z/V, so the work is fully independent across
(b, h): 32 tasks, 4 per core, no collectives.

Per-core program v2 (PE-lean + 3-engine softmax):
  - projections in fp8e4 DoubleRow with hi/lo compensation (3 products
    hh+hl+lh per k-chunk; lo*lo dropped).  X scaled x8, W x16 on the host;
    the 128x product scale folds into the exp scale (2^-17) and the ones
    block (128.0), so all drains are plain copies.
  - scores fp16 (error budget: fp8 scores measured 8e-2, fp16 1e-3).
  - exp split across engines: ACT does exact exp for 12/16 jp chunks;
    the rest run on DVE as Schraudolph bits (tensor_scalar -> int32:
    bits = t*2^23 + 127*2^23, giving 2^floor(t)*(1+frac(t)) as the fp32
    view) followed by a pair-batched 1-instruction custom DVE op applying
    the g(u) = 1 + a*u*(1-u) correction via BITWISE_AND/OR mantissa
    extraction (max rel err 5.2e-3, washes to ~3e-3 on the final output).
    GPSIMD cannot read PSUM, so it only gets the bo broadcast.
  - A@V fp16 with a 64-wide ones*128 block -> 64 denominator copies on
    partitions 64..127 (stationary free size is not charged by the PE).
  - normalize: one reciprocal + one [64,4,128] multiply per half-block.
  - out = Hcat @ Wo.T fp16 + b_o, fp16 DMA out (host casts to f32).
    K projection is emitted interleaved with the first block's score
    stream; deferred outproj matmuls/drains slot into the next block at
    steps OP_MM_STEP/OP_DR_STEP.
"""

import numpy as np

import concourse.bass as bass
import concourse.mybir as mybir
import concourse.tile as tile
from concourse import bacc

B, S, D = 2, 2048, 1024
H, DK = 16, 64
NCORES = 8
HPC = H // (NCORES // B)  # heads per core = 4
SC = HPC * 128            # s-rows per core = 512
P = 128
KD = D // P               # 8 contraction chunks
PO = D // P               # 8 feature chunks
F32 = mybir.dt.float32
F16 = mybir.dt.float16
F8 = mybir.dt.float8e4
I32 = mybir.dt.int32
MODE = "v2"

XS, WS = 8.0, 16.0        # host-side fp8 hi/lo scaling (subnormal avoidance)
ONES = float(XS * WS)     # folded product scale -> ones block / exp scale
EXP_SCALE = 0.125 / (XS * WS * XS * WS)   # = 2^-17, exact
C1S = float(np.log2(np.e) * EXP_SCALE * (1 << 23))   # schraudolph mult
C2S = float(127 * (1 << 23))                         # schraudolph bias
EXP_A = -0.235490                                    # g = 1 + a*u*(1-u)
MASK_F = float(np.int32(0x007FFFFF).view(np.float32))

# jp chunks whose exp runs on ACT (rest: DVE bits + DVE correction).
ACT_JP = frozenset(jp for jp in range(16) if jp not in (3, 6, 9, 12))
LAG = 3                   # jp lag between scores/exp and A@V consumption
OP_MM_STEP = 14           # next-block step at which deferred outproj matmuls run
OP_DR_STEP = 15           # next-block step at which outproj drains run


def register_exp_op():
    """Register the custom DVE exp-correction op (idempotent)."""
    import concourse.dve_ops as dve_ops
    from concourse.dve_spec import AluOp, Bin, One, Spec, Src0, C0, C1, \
        lower, _has_src1
    from concourse.dve_uop import DveOpSpec

    name = "EXP2_CORR_ANT"
    if name in dve_ops.CUSTOM_DVE_SPECS:
        return (next(op for op in dve_ops.OPS if op.name == name),
                next(op for op in dve_ops.OPS if op.name == "NORM_RECIP_ANT"))

    n = Bin(AluOp.BITWISE_AND, Src0, C0)   # mantissa bits of 2^i*(1+u)
    y1 = Bin(AluOp.BITWISE_OR, n, One)     # 1+u as a normal float
    u = y1 - One
    body = (One + (u * (One - u)) * C1) * Src0

    def ref(in0, in1, s0, s1, imm2):
        b = np.ascontiguousarray(np.asarray(in0, np.float32)).view(np.int32)
        y1r = ((b & 0x007FFFFF) | 0x3F800000).view(np.float32)
        ur = y1r - 1.0
        return ((1.0 + s1 * ur * (1.0 - ur))
                * np.asarray(in0, np.float32)).astype(np.float32)

    spec = Spec(body=body, reference=ref)

    def reg(name, spec):
        row = max(dve_ops._SUB_OPCODE_FOR_NAME.values()) + 1
        assert row < 0x20
        uops = lower(spec, ver="v3")
        sha = DveOpSpec(name=name, opcode=row, uops=uops,
                        rd1_en=_has_src1(spec)).sha("v3")
        op = dve_ops.DveOp(name, spec, subdim=False, uops_sha={"v3": sha})
        dve_ops._SUB_OPCODE_FOR_NAME[name] = row
        dve_ops.OPS.append(op)
        dve_ops.CUSTOM_DVE_SPECS[name] = spec
        return op

    op_exp = reg(name, spec)

    # normalize: out = Src0 * approx(1/Src1); BITWISE_NOT exponent-flip
    # seed + one Chebyshev-scaled NR pass (rel err ~2e-3; see
    # RECIPROCAL_APPROX_FAST in dve_ops for the constant derivation)
    from concourse.dve_spec import Src1
    nx = Bin(AluOp.BITWISE_NOT, Src1, Src1)
    y0 = nx * C0
    y1 = y0 * (C1 - Src1 * y0)
    nbody = y1 * Src0

    def nref(in0, in1, s0, s1, imm2):
        nxr = (~np.ascontiguousarray(np.asarray(in1, np.float32))
               .view(np.int32)).view(np.float32)
        y0r = nxr * s0
        y1r = y0r * (s1 - np.asarray(in1, np.float32) * y0r)
        return (y1r * np.asarray(in0, np.float32)).astype(np.float32)

    op_norm = reg("NORM_RECIP_ANT", Spec(body=nbody, reference=nref))
    return op_exp, op_norm


def build_body(nc, out_ap, ins):
    """Emit the per-core program. ins: dict of DRAM APs."""
    exp_op, norm_op = register_exp_op()
    EXPF = mybir.ActivationFunctionType.Exp
    MULT = mybir.AluOpType.mult
    ADD = mybir.AluOpType.add
    DR = mybir.MatmulPerfMode.DoubleRow

    with tile.TileContext(nc) as tc:
        with (
            tc.tile_pool(name="singles", bufs=1) as singles,
            tc.tile_pool(name="wp", bufs=2) as wp,
            tc.tile_pool(name="xp", bufs=2) as xp,
            tc.tile_pool(name="exp", bufs=16) as exp_pool,
            tc.tile_pool(name="bits", bufs=3) as bits_pool,
            tc.tile_pool(name="smalls", bufs=3) as smalls,
            tc.tile_pool(name="outp", bufs=4) as outp,
            tc.tile_pool(name="ps_mm", bufs=2, space="PSUM") as ps_mm,
            tc.tile_pool(name="ps_at", bufs=4, space="PSUM") as ps_at,
            tc.tile_pool(name="ps_o", bufs=2, space="PSUM") as ps_o,
        ):
            # --- constants ---
            bo_sb = singles.tile([P, D], F32, tag="bo", name="bo_sb")

            qt_sb = singles.tile([P, PO, SC], F16, tag="qt", name="qt_sb")
            kt_sb = singles.tile([P, PO, SC], F16, tag="kt", name="kt_sb")
            # partition-rotated copy: kt2[pi] = kt[(pi+64) % 128] so K slices
            # of either parity sit at either partition base
            kt2_sb = singles.tile([P, PO, SC], F16, tag="kt2", name="kt2_sb")
            hcat = singles.tile([P, PO, SC], F16, tag="hcat", name="hcat")
            # [V*128 | ones*128]: A@V emits 64 denominator copies on
            # partitions 64..127 (stationary free size costs nothing)
            v_sb = [singles.tile([P, 16, P], F16, tag=f"v{hl}",
                                 name=f"v_sb{hl}") for hl in range(HPC)]
            for hl in range(HPC):
                nc.vector.memset(v_sb[hl][:, :, DK:P], ONES)



            # --- loads ---
            # W tensors arrive in f-halves (each mf-block needs only its
            # f-window but every kd chunk); X tensors in kd-quarters (each
            # mf-block needs all of X).  Per projection: w_hi fh, x_hi,
            # x_lo, w_lo fh, then the second f-half pair.
            def load_w8(pool, tag, ap_h, ap_l):
                wh = pool.tile([P, KD, D], F8, tag=f"{tag}h", name=f"{tag}h")
                wl = pool.tile([P, KD, D], F8, tag=f"{tag}l", name=f"{tag}l")
                srcs = (ap_h.rearrange("(kd p) f -> p kd f", p=P),
                        ap_l.rearrange("(kd p) f -> p kd f", p=P))

                def fh(i, h):
                    t, s = (wh, srcs[0]) if i == 0 else (wl, srcs[1])
                    nc.sync.dma_start(t[:, :, h * 512:(h + 1) * 512],
                                      s[:, :, h * 512:(h + 1) * 512])
                return wh, wl, fh

            def load_x8(tag, ap_h, ap_l):
                xh = xp.tile([P, KD, SC], F8, tag="xh", name=f"{tag}h")
                xl = xp.tile([P, KD, SC], F8, tag="xl", name=f"{tag}l")
                for t, ap in ((xh, ap_h), (xl, ap_l)):
                    s = ap.rearrange("(kd p) f -> p kd f", p=P)
                    for q in range(0, KD, 2):
                        nc.sync.dma_start(t[:, q:q + 2], s[:, q:q + 2])
                return xh, xl

            def load_proj(pool, wtag, nm):
                wh, wl, fh = load_w8(pool, wtag, ins[f"w{nm}h"],
                                     ins[f"w{nm}l"])
                fh(0, 0)
                xh, xl = load_x8(f"x{nm}", ins[f"x{nm}h"], ins[f"x{nm}l"])
                fh(1, 0)
                fh(0, 1)
                fh(1, 1)
                return wh, wl, xh, xl

            # 3-product compensated fp8 DoubleRow accumulation:
            # psum += Xhi@Whi + Xhi@Wlo + Xlo@Whi over KD k-chunks, k-chunk
            # pairs packed two per DoubleRow matmul (24 products -> 12 mm).
            def dr3(ps, lhs_hi, lhs_lo, rhs_hi, rhs_lo, lf, rf):
                prods = [(lhs_hi, rhs_hi), (lhs_hi, rhs_lo), (lhs_lo, rhs_hi)]
                n = len(prods) * (KD // 2)
                i = 0
                for lt, rt in prods:
                    for kd in range(0, KD, 2):
                        nc.tensor.matmul(
                            ps, lt[:, kd:kd + 2, lf], rt[:, kd:kd + 2, rf],
                            start=(i == 0), stop=(i == n - 1), perf_mode=DR)
                        i += 1

            # --- Q projection (transposed): QT[f, s] ---
            wq_hi, wq_lo, xq_hi, xq_lo = load_proj(wp, "w", "q")

            def q_proj(mf):
                ps = ps_mm.tile([P, SC], F32, tag="mm", name="ps")
                dr3(ps, wq_hi, wq_lo, xq_hi, xq_lo,
                    slice(mf * P, (mf + 1) * P), slice(None))
                nc.vector.tensor_copy(qt_sb[:, mf, :], ps)

            for mf in range(PO):
                q_proj(mf)

            # --- K then V loads; K projection rides the first block ---
            wk_hi, wk_lo, xk_hi, xk_lo = load_proj(wp, "w", "k")
            wv_hi, wv_lo, xv_hi, xv_lo = load_proj(wp, "w", "v")

            def v_proj(hl):
                for nf in range(2):
                    ps = ps_mm.tile([P, SC], F32, tag="mm", name="ps")
                    dr3(ps, xv_hi, xv_lo, wv_hi, wv_lo,
                        slice(hl * P, (hl + 1) * P),
                        slice(nf * 512, (nf + 1) * 512))
                    nc.vector.tensor_copy(
                        v_sb[hl][:, nf * 8:(nf + 1) * 8, 0:DK],
                        ps.rearrange("p (j k) -> p j k", k=DK))

            def k_proj(mf):
                ps = ps_mm.tile([P, SC], F32, tag="mm", name="ps")
                dr3(ps, wk_hi, wk_lo, xk_hi, xk_lo,
                    slice(mf * P, (mf + 1) * P), slice(None))
                nc.vector.tensor_copy(kt_sb[:, mf, :], ps)
                nc.sync.dma_start(kt2_sb[0:64, mf], kt_sb[64:128, mf])
                nc.sync.dma_start(kt2_sb[64:128, mf], kt_sb[0:64, mf])

            pending = []

            def make_block(hl, pp):
                hs = slice(hl * P, (hl + 1) * P)
                rhs_a = qt_sb[0:64, 4 * pp:4 * pp + 4, hs]
                rhs_b = qt_sb[64:128, 4 * pp:4 * pp + 4, hs]
                ost = {}
                exq = []

                def o_tiles():
                    if not ost:
                        ost["a"] = ps_o.tile([P, 512], F32, tag="o",
                                             name="o_a")
                        ost["b"] = ps_o.tile([P, 512], F32, tag="o",
                                             name="o_b")
                    return ost["a"], ost["b"]

                def at_fn(jp):
                    if True:
                        a2, po2 = jp % 2, jp // 2
                        ksrc_a = kt_sb if a2 == 0 else kt2_sb
                        ksrc_b = kt_sb if a2 == 1 else kt2_sb
                        at_a = ps_at.tile([P, 512], F32, tag="at",
                                          name="at_a")
                        at_b = ps_at.tile([P, 512], F32, tag="at",
                                          name="at_b")
                        nc.tensor.matmul(at_a, ksrc_a[0:64, po2, hs],
                                         rhs_a, start=True, stop=True)
                        nc.tensor.matmul(at_b, ksrc_b[64:128, po2, hs],
                                         rhs_b, start=True, stop=True)
                        ex = exp_pool.tile([P, 2, 512], F16, tag="ex",
                                           name="ex")
                        if jp in ACT_JP:
                            # one ACTIVATE per PSUM bank (2-bank reads
                            # hang the device)
                            nc.scalar.activation(ex[:, 0], at_a, EXPF,
                                                 scale=EXP_SCALE)
                            nc.scalar.activation(ex[:, 1], at_b, EXPF,
                                                 scale=EXP_SCALE)
                        else:
                            # fastexp: schraudolph bits on DVE (one
                            # tensor_scalar per PSUM bank), then the
                            # pair-batched custom correction op
                            bt = bits_pool.tile([P, 2, 512], I32,
                                                tag="bt", name="bt")
                            nc.vector.tensor_scalar(bt[:, 0], at_a,
                                                    C1S, C2S, MULT, ADD)
                            nc.vector.tensor_scalar(bt[:, 1], at_b,
                                                    C1S, C2S, MULT, ADD)
                            nc.vector._custom_dve(
                                exp_op, out=ex, in0=bt.bitcast(F32),
                                s0=MASK_F, s1=EXP_A)
                        exq.append(ex)

                def o_fn(jq):
                    o_a, o_b = o_tiles()
                    st, sp = jq == 0, jq == 15
                    ex = exq[jq]
                    nc.tensor.matmul(o_a, v_sb[hl][:, jq, :],
                                     ex[:, 0], start=st, stop=sp)
                    nc.tensor.matmul(o_b, v_sb[hl][:, jq, :],
                                     ex[:, 1], start=st, stop=sp)

                def finish_fn():
                    # normalize into HcatT: 64 denominator copies ->
                    # one reciprocal + one multiply per half
                    for a, o_ps in ((0, ost["a"]), (1, ost["b"])):
                        rc = smalls.tile([P, 512], F32, tag="rc", name="rc")
                        nc.vector.reciprocal(rc[64:128, :], o_ps[64:128, :])
                        nc.vector.tensor_tensor(
                            hcat[64 * a:64 * a + 64, 4 * pp:4 * pp + 4, hs],
                            o_ps[0:64, :].rearrange("k (c t) -> k c t", t=P),
                            rc[64:128, :].rearrange("k (c t) -> k c t", t=P),
                            MULT)

                return at_fn, o_fn, finish_fn, o_tiles

            def outproj_mm(hl, po0, po1, pss=None):
                hs = slice(hl * P, (hl + 1) * P)
                if pss is None:
                    pss = [ps_mm.tile([P, 512], F32, tag="mm", name="ps")
                           for _ in range(2)]

                for nf in range(2):
                    fs = slice(nf * 512, (nf + 1) * 512)
                    for po in range(po0, po1):
                        nc.tensor.matmul(
                            pss[nf], hcat[:, po, hs], wo_t[:, po, fs],
                            start=(po == 0), stop=(po == PO - 1))
                return pss

            def outproj_drain(hl, pss):
                hs = slice(hl * P, (hl + 1) * P)
                for nf in range(2):
                    fs = slice(nf * 512, (nf + 1) * 512)
                    os_t = outp.tile([P, 512], F16, tag="os", name="os_t")
                    nc.vector.tensor_tensor(os_t, pss[nf], bo_sb[:, fs], ADD)
                    nc.sync.dma_start(out_ap[hs, fs], os_t)

            # first block rides the K projection: emit K mf-chunk m+1
            # while its scores (jp 2m, 2m+1) run; the A@V stream waits for
            # v_proj(0), which lands right after the score stream
            at0, o0, fin0, _ = make_block(0, 0)
            k_proj(0)
            for m in range(1, PO):
                k_proj(m)
                at0(2 * (m - 1))
                at0(2 * (m - 1) + 1)
            at0(14)
            at0(15)
            v_proj(0)
            for jq in range(16):
                o0(jq)
            fin0()
            for h2 in range(1, HPC):
                v_proj(h2)

            bo = ins["bo"]
            bo_bcast = bass.AP(tensor=bo.tensor, offset=bo.offset,
                               ap=[[0, P], list(bo.ap[-1])])
            nc.gpsimd.dma_start(out=bo_sb, in_=bo_bcast)

            def load_w16(ap):
                t = singles.tile([P, KD, D], F16, tag="wo", name="wo_t")
                s = ap.rearrange("(kd p) f -> p kd f", p=P)
                for q in range(0, KD, 2):
                    nc.sync.dma_start(t[:, q:q + 2], s[:, q:q + 2])
                return t

            wo_t = load_w16(ins["wot"])  # prefetched during attention

            op_state = {}
            for blk in range(1, 2 * HPC):
                hl, pp = divmod(blk, 2)
                last = blk == 2 * HPC - 1
                at_fn, o_fn, finish_fn, o_tiles = make_block(hl, pp)
                for s in range(16 + LAG):
                    if s == OP_MM_STEP and "mm" in op_state:
                        h2 = op_state.pop("mm")
                        op_state["dr"] = (h2, outproj_mm(h2, 0, PO))
                    if s == OP_DR_STEP and "dr" in op_state:
                        h2, pss = op_state.pop("dr")
                        outproj_drain(h2, pss)
                    if s < 16:
                        at_fn(s)
                    if s >= LAG:
                        o_fn(s - LAG)
                finish_fn()
                if pp == 1 and not last:
                    op_state["mm"] = hl
            pss = outproj_mm(3, 0, PO)
            outproj_drain(3, pss)

            for emit in pending:
                emit()
    return nc


def build_program():
    nc = bacc.Bacc("TRN2", target_bir_lowering=False, debug=False,
                   enable_asserts=False, num_devices=NCORES)
    ins = {}
    for nm in ("q", "k", "v"):
        for hx in ("h", "l"):
            ins[f"w{nm}{hx}"] = nc.dram_tensor(
                f"w{nm}{hx}", [D, D], F8, kind="ExternalInput").ap()
            ins[f"x{nm}{hx}"] = nc.dram_tensor(
                f"x{nm}{hx}", [D, SC], F8, kind="ExternalInput").ap()
    ins["wot"] = nc.dram_tensor("wot", [D, D], F16, kind="ExternalInput").ap()
    ins["bo"] = nc.dram_tensor("bo", [1, D], F32, kind="ExternalInput").ap()
    out_ap = nc.dram_tensor("out", [SC, D], F16, kind="ExternalOutput").ap()
    build_body(nc, out_ap, ins)
    nc.finalize()
    return nc


def _split8(x, scale):
    """x (f32) -> (hi, lo) fp8e4 pair of x*scale."""
    import ml_dtypes
    xs = np.ascontiguousarray(x * scale)
    hi = xs.astype(ml_dtypes.float8_e4m3)
    lo = (xs - hi.astype(np.float32)).astype(ml_dtypes.float8_e4m3)
    return hi, lo


def make_in_maps(inputs):
    import ml_dtypes
    Xs = {n: np.asarray(inputs[k], np.float32)
          for n, k in (("q", "X_q"), ("k", "X_k"), ("v", "X_v"))}
    common = {}
    for n, k in (("q", "W_q"), ("k", "W_k"), ("v", "W_v")):
        wt = np.ascontiguousarray(np.asarray(inputs[k], np.float32).T)
        common[f"w{n}h"], common[f"w{n}l"] = _split8(wt, WS)
    common["wot"] = np.ascontiguousarray(
        np.asarray(inputs["W_o"], np.float32).T).astype(np.float16)
    common["bo"] = np.asarray(inputs["b_o"], np.float32).reshape(1, D)

    xt = {n: [np.ascontiguousarray(x[b].T) for b in range(B)]
          for n, x in Xs.items()}
    in_maps = []
    for c in range(NCORES):
        b, g = divmod(c, NCORES // B)
        sl = slice(g * SC, (g + 1) * SC)
        m = dict(common)
        for n in ("q", "k", "v"):
            hi, lo = _split8(np.ascontiguousarray(xt[n][b][:, sl]), XS)
            m[f"x{n}h"], m[f"x{n}l"] = hi, lo
        in_maps.append(m)
    return in_maps


_NC_CACHE = {}


def _run(inputs, trace=False, trace_cores=None):
    from concourse.bass_utils import run_bass_kernel_spmd
    if MODE not in _NC_CACHE:
        _NC_CACHE[MODE] = build_program()
    nc = _NC_CACHE[MODE]
    in_maps = make_in_maps(inputs)
    res = run_bass_kernel_spmd(nc, in_maps, core_ids=list(range(NCORES)),
                               trace=trace, trace_cores=trace_cores)
    out = np.empty((B, S, D), dtype=np.float32)
    for c in range(NCORES):
        b, g = divmod(c, NCORES // B)
        out[b, g * SC:(g + 1) * SC, :] = res.results[c]["out"].astype(np.float32)
    return out, res


def kernel(**inputs):
    out, _ = _run(inputs, trace=False)
    return out
